# revision 19
# baseline (speedup 1.0000x reference)
"""Trainium2 Bass kernel for DiceLoss (hard-argmax dice, ignore background, mean).

Problem (hardcoded shapes):
  y_true: [16, 512, 512] int32 in [0, 8)
  y_pred: [16, 8, 512, 512] float32
  out   : scalar float32 = mean over classes 1..7 of
          (2*tp + eps) / (2*tp + fp + fn + eps)
  with pred_cls = argmax_c y_pred, one-hot tp/fp/fn sums over all pixels.
  Note 2*tp + fp + fn == pred_cnt + gt_cnt, so per class we only need
  tp, pred_cnt (both from the device) and gt_cnt (host bincount of y_true).

Strategy (8 NeuronCores, data-parallel over batch; measured-on-HW facts in
brackets):
  - Each core processes 2 of the 16 batch images (SPMD, same NEFF), streamed
    in 4 chunks of [128, 1024] per plane (contiguous HBM -> peak DMA).
  - ScalarE (ACT, otherwise idle): converts the 8 channel planes fp32->fp16
    and the label plane int32->fp16 [~1.04us per [128,1024] op].
  - VectorE (DVE): everything in fp16 to hit the hardware perf modes
    [tensor_tensor 16-bit in+out runs 2x (~600ns); tensor_scalar 16-bit
    without accum_out runs 4x (~335ns); accum_out forces 1x (~1130ns) so it
    is avoided entirely]:
      * 7-op pairwise-max tree over the fp16 channels -> m
      * pred_c = tensor_tensor is_equal(ch_fp16[c], m)   (7 ops @ 2x)
      * gt_c   = tensor_scalar is_equal(lab_fp16, c)     (7 ops @ 4x)
    fp16 (10-bit mantissa) keeps argmax-tie inflation ~0.1% of pixels,
    far inside the 2e-2 tolerance.
  - TensorE (PE): per class, tp via diag(pred_c^T @ gt_c) accumulated in a
    [128,128] PSUM over subtiles+chunks, plus pred_cnt via an extra
    rhs=ones[128,1] matmul on the already-loaded pred_c weights
    [LDWEIGHTS and MATMUL pipeline on separate units, so the extra
    matmuls ride along under the DMA roofline].
  - Host: gt counts from np.bincount(y_true) (exact, input-only), then the
    dice mean in float32 mirroring the reference arithmetic.
"""

import numpy as np

EPS = 1e-05

# Problem geometry (hardcoded per the harness contract).
N_CORES = 8
NB = 2          # batch images per core
C = 8           # classes
P = 128         # SBUF partitions
F = 1024        # free-dim elements per chunk
NCHUNK = 2      # chunks per image plane (512*512 = 2*128*1024)
CHUNKS = NB * NCHUNK
NSUB = F // 128  # 128-wide subtiles per chunk for the PE tp matmuls

_CACHED_NC = None


def build_bass():
    """Build the Bass kernel (same NEFF for all 8 cores)."""
    from contextlib import ExitStack

    import concourse.bacc as bacc
    import concourse.tile as tile
    from concourse import mybir

    nc = bacc.Bacc(None, target_bir_lowering=False)
    f32 = mybir.dt.float32
    fp16 = mybir.dt.float16
    i32 = mybir.dt.int32
    A = mybir.AluOpType

    i16 = mybir.dt.int16

    yp = nc.dram_tensor(
        "yp", [NB, C, NCHUNK, P, F], f32, kind="ExternalInput"
    )
    # labels as little-endian int16 pairs: [..., 0] is the value (labels < 8),
    # letting the DMA deliver 16-bit labels that compare at DVE 4x perf mode
    yt = nc.dram_tensor("yt", [NB, NCHUNK, P, F, 2], i16, kind="ExternalInput")
    # tp partials: per class a [128, 128] PSUM accumulator; host takes trace().
    tp_out = nc.dram_tensor("tp_out", [P, 7, 128], f32, kind="ExternalOutput")
    # pred counts: per class a [128, 1] PSUM accumulator; host sums partitions.
    pc_out = nc.dram_tensor("pc_out", [P, 7], f32, kind="ExternalOutput")

    with tile.TileContext(nc) as tc, ExitStack() as ctx:
        chpool = ctx.enter_context(tc.tile_pool(name="ch", bufs=2))
        hpool = ctx.enter_context(tc.tile_pool(name="h", bufs=2))
        tpool = ctx.enter_context(tc.tile_pool(name="tt", bufs=2))
        mpool = ctx.enter_context(tc.tile_pool(name="mx", bufs=2))
        mtmp = ctx.enter_context(tc.tile_pool(name="mtmp", bufs=6))
        maskp = ctx.enter_context(tc.tile_pool(name="mask", bufs=3))
        gtpool = ctx.enter_context(tc.tile_pool(name="gt", bufs=3))
        constp = ctx.enter_context(tc.tile_pool(name="const", bufs=1))
        accp = ctx.enter_context(tc.tile_pool(name="acc", bufs=1))
        psump = ctx.enter_context(tc.tile_pool(name="psum", bufs=1, space="PSUM"))

        ones = constp.tile([P, 1], fp16, name="ones")
        nc.vector.memset(ones, 1.0)

        # one 7-bank PSUM tile: class c's [128,128] tp accumulator lives at
        # the start of bank c-1 (512-f32 bank stride), so start=True bank
        # resets stay per-class, and the final drain is one strided copy.
        BANK = 512
        tpbank = psump.tile([P, 7, BANK], f32, name="tpbank", tag="tpbank")
        psums = [tpbank[:, c - 1, 0:128] for c in range(1, C)]
        # all 7 pred-count accumulators share the 8th PSUM bank
        cntbank = psump.tile([P, 8], f32, name="cntbank", tag="cntbank")
        cnts = [cntbank[:, c - 1 : c] for c in range(1, C)]

        chunk_idx = 0
        for n in range(NB):
            for j in range(NCHUNK):
                ch = []
                for c in range(C):
                    tl = chpool.tile([P, F], f32, name=f"ch{c}", tag=f"ch{c}")
                    nc.sync.dma_start(out=tl, in_=yp[n, c, j])
                    ch.append(tl)
                tf = tpool.tile([P, F], i16, name="tf", tag="tf")
                # strided low-half loads; split so no flattened AP count
                # exceeds the 16-bit ISA descriptor field
                for q in range(4):
                    nc.sync.dma_start(
                        out=tf[32 * q : 32 * (q + 1), :],
                        in_=yt[n, j, 32 * q : 32 * (q + 1), :, 0],
                    )

                # ScalarE: fp32 -> fp16 channel converts
                chf = []
                for c in range(C):
                    tl = hpool.tile([P, F], fp16, name=f"hf{c}", tag=f"hf{c}")
                    nc.scalar.copy(out=tl, in_=ch[c])
                    chf.append(tl)

                # ---- max tree (DVE tensor_tensor fp16: 2x perf mode) ----
                m01 = mtmp.tile([P, F], fp16, name="m01", tag="mt")
                nc.vector.tensor_max(m01, chf[0], chf[1])
                m23 = mtmp.tile([P, F], fp16, name="m23", tag="mt")
                nc.vector.tensor_max(m23, chf[2], chf[3])
                m45 = mtmp.tile([P, F], fp16, name="m45", tag="mt")
                nc.vector.tensor_max(m45, chf[4], chf[5])
                m67 = mtmp.tile([P, F], fp16, name="m67", tag="mt")
                nc.vector.tensor_max(m67, chf[6], chf[7])
                m0123 = mtmp.tile([P, F], fp16, name="m0123", tag="mt")
                nc.vector.tensor_max(m0123, m01, m23)
                m4567 = mtmp.tile([P, F], fp16, name="m4567", tag="mt")
                nc.vector.tensor_max(m4567, m45, m67)
                m = mpool.tile([P, F], fp16, name="m", tag="m")
                nc.vector.tensor_max(m, m0123, m4567)

                # ---- per-class masks + PE tp/count matmuls ----
                for c in range(1, C):
                    pred = maskp.tile([P, F], fp16, name=f"pred{c}", tag="pred")
                    nc.vector.tensor_tensor(pred, chf[c], m, A.is_equal)
                    gt = gtpool.tile([P, F], fp16, name=f"gt{c}", tag="gt")
                    nc.vector.tensor_scalar(gt, tf, c, None, A.is_equal)
                    for s in range(NSUB):
                        first = chunk_idx == 0 and s == 0
                        last = chunk_idx == CHUNKS - 1 and s == NSUB - 1
                        nc.tensor.matmul(
                            psums[c - 1],
                            lhsT=pred[:, s * 128 : (s + 1) * 128],
                            rhs=gt[:, s * 128 : (s + 1) * 128],
                            start=first,
                            stop=last,
                            skip_group_check=True,
                        )
                        # cnts share one PSUM bank and start=True resets the
                        # WHOLE bank (measured): only the very first count
                        # matmul may use it; all later chains accumulate.
                        nc.tensor.matmul(
                            cnts[c - 1],
                            lhsT=pred[:, s * 128 : (s + 1) * 128],
                            rhs=ones[:, :],
                            start=first and c == 1,
                            stop=last and c == C - 1,
                            skip_group_check=True,
                        )
                chunk_idx += 1

        # drain: one strided copy picks the [128,128] block out of each bank
        tps = accp.tile([P, 7, 128], f32, name="tps")
        nc.scalar.copy(out=tps, in_=tpbank[:, :, 0:128])
        nc.sync.dma_start(out=tp_out[:, :, :], in_=tps)
        pcs = accp.tile([P, 7], f32, name="pcs")
        nc.scalar.copy(out=pcs, in_=cntbank[:, 0:7])
        nc.sync.dma_start(out=pc_out[:, :], in_=pcs)

    nc.finalize()
    return nc


def _get_bass():
    global _CACHED_NC
    if _CACHED_NC is None:
        _CACHED_NC = build_bass()
    return _CACHED_NC


def make_in_maps(y_true, y_pred):
    yp = np.ascontiguousarray(np.asarray(y_pred, dtype=np.float32))
    yt = np.ascontiguousarray(np.asarray(y_true, dtype=np.int32))
    yt16 = yt.view(np.int16)  # little-endian: [..., 0] is the low half-word
    in_maps = []
    for i in range(N_CORES):
        yps = np.ascontiguousarray(yp[NB * i : NB * (i + 1)]).reshape(NB, C, NCHUNK, P, F)
        yts = np.ascontiguousarray(yt16[NB * i : NB * (i + 1)]).reshape(
            NB, NCHUNK, P, F, 2
        )
        in_maps.append({"yp": yps, "yt": yts})
    return in_maps


def epilogue(results, y_true):
    """Combine the 8 cores' partial sums into the final dice mean (float32,
    mirroring the reference arithmetic). gt counts come from the labels
    directly (exact)."""
    tp = np.zeros(7, dtype=np.float64)
    pred_cnt = np.zeros(7, dtype=np.float64)
    for r in results:
        # tp_out[m, c, n] = psum_c[m, n]; tp_c = sum_m psum_c[m, m]
        tp += np.einsum("mcm->c", np.asarray(r["tp_out"], dtype=np.float64))
        pred_cnt += np.asarray(r["pc_out"], dtype=np.float64).sum(axis=0)
    gt_cnt = np.bincount(
        np.asarray(y_true, dtype=np.int64).ravel(), minlength=8
    )[1:].astype(np.float64)

    # dice = (2tp + eps) / (2tp + fp + fn + eps), and
    # 2tp + fp + fn = pred_cnt + gt_cnt
    tp32 = tp.astype(np.float32)
    denom = (pred_cnt + gt_cnt).astype(np.float32)
    eps = np.float32(EPS)
    two = np.float32(2.0)
    dice = (two * tp32 + eps) / (denom + eps)
    return np.asarray(np.mean(dice, dtype=np.float32), dtype=np.float32)


def kernel(**inputs):
    from concourse.bass_utils import run_bass_kernel_spmd

    nc = _get_bass()
    in_maps = make_in_maps(inputs["y_true"], inputs["y_pred"])
    res = run_bass_kernel_spmd(nc, in_maps, core_ids=list(range(N_CORES)))
    return epilogue(res.results, inputs["y_true"])


if __name__ == "__main__":
    # smoke test with random data
    rng = np.random.default_rng(0)
    y_true = rng.integers(0, C, size=(16, 512, 512)).astype(np.int32)
    y_pred = rng.standard_normal((16, C, 512, 512)).astype(np.float32)
    out = kernel(y_true=y_true, y_pred=y_pred)
    print("kernel output:", out)


# revision 24
# speedup vs baseline: 11.9119x; 11.9119x over previous
"""Trainium2 Bass kernel for DiceLoss (hard-argmax dice, ignore background, mean).

Problem (hardcoded shapes):
  y_true: [16, 512, 512] int32 in [0, 8)
  y_pred: [16, 8, 512, 512] float32
  out   : scalar float32 = mean over classes 1..7 of
          (2*tp + eps) / (2*tp + fp + fn + eps)
  with pred_cls = argmax_c y_pred, one-hot tp/fp/fn sums over all pixels.
  Note 2*tp + fp + fn == pred_cnt + gt_cnt, so per class we only need
  tp, pred_cnt (both from the device) and gt_cnt (host bincount of y_true).

Strategy (8 NeuronCores, data-parallel over batch; measured-on-HW facts in
brackets):
  - Each core processes 2 of the 16 batch images (SPMD, same NEFF), streamed
    in 4 chunks of [128, 1024] per plane (contiguous HBM -> peak DMA).
  - ScalarE (ACT, otherwise idle): converts the 8 channel planes fp32->fp16
    and the label plane int32->fp16 [~1.04us per [128,1024] op].
  - VectorE (DVE): everything in fp16 to hit the hardware perf modes
    [tensor_tensor 16-bit in+out runs 2x (~600ns); tensor_scalar 16-bit
    without accum_out runs 4x (~335ns); accum_out forces 1x (~1130ns) so it
    is avoided entirely]:
      * 7-op pairwise-max tree over the fp16 channels -> m
      * pred_c = tensor_tensor is_equal(ch_fp16[c], m)   (7 ops @ 2x)
      * gt_c   = tensor_scalar is_equal(lab_fp16, c)     (7 ops @ 4x)
    fp16 (10-bit mantissa) keeps argmax-tie inflation ~0.1% of pixels,
    far inside the 2e-2 tolerance.
  - TensorE (PE): per class, tp via diag(pred_c^T @ gt_c) accumulated in a
    [128,128] PSUM over subtiles+chunks, plus pred_cnt via an extra
    rhs=ones[128,1] matmul on the already-loaded pred_c weights
    [LDWEIGHTS and MATMUL pipeline on separate units, so the extra
    matmuls ride along under the DMA roofline].
  - Host: gt counts from np.bincount(y_true) (exact, input-only), then the
    dice mean in float32 mirroring the reference arithmetic.
"""

import numpy as np

EPS = 1e-05

# Problem geometry (hardcoded per the harness contract).
N_CORES = 8
NB = 2          # batch images per core
C = 8           # classes
P = 128         # SBUF partitions
F = 1024        # free-dim elements per chunk
NCHUNK = 2      # chunks per image plane (512*512 = 2*128*1024)
CHUNKS = NB * NCHUNK
NSUB = F // 128  # 128-wide subtiles per chunk for the PE tp matmuls

_CACHED_NC = None


def build_bass():
    """Build the Bass kernel (same NEFF for all 8 cores)."""
    from contextlib import ExitStack

    import concourse.bacc as bacc
    import concourse.tile as tile
    from concourse import mybir

    nc = bacc.Bacc(None, target_bir_lowering=False)
    f32 = mybir.dt.float32
    fp16 = mybir.dt.float16
    i32 = mybir.dt.int32
    A = mybir.AluOpType

    yp = nc.dram_tensor(
        "yp", [NB, C, NCHUNK, P, F], f32, kind="ExternalInput"
    )
    yt = nc.dram_tensor("yt", [NB, NCHUNK, P, F], i32, kind="ExternalInput")
    # tp partials: per class a [128, 128] PSUM accumulator; host takes trace().
    tp_out = nc.dram_tensor("tp_out", [P, 7, 128], f32, kind="ExternalOutput")
    # pred counts: per class a [128, 1] PSUM accumulator; host sums partitions.
    pc_out = nc.dram_tensor("pc_out", [P, 7], f32, kind="ExternalOutput")

    with tile.TileContext(nc) as tc, ExitStack() as ctx:
        chpool = ctx.enter_context(tc.tile_pool(name="ch", bufs=2))
        hpool = ctx.enter_context(tc.tile_pool(name="h", bufs=2))
        tpool = ctx.enter_context(tc.tile_pool(name="tt", bufs=2))
        mpool = ctx.enter_context(tc.tile_pool(name="mx", bufs=2))
        mtmp = ctx.enter_context(tc.tile_pool(name="mtmp", bufs=6))
        maskp = ctx.enter_context(tc.tile_pool(name="mask", bufs=3))
        gtpool = ctx.enter_context(tc.tile_pool(name="gt", bufs=9))
        constp = ctx.enter_context(tc.tile_pool(name="const", bufs=1))
        accp = ctx.enter_context(tc.tile_pool(name="acc", bufs=1))
        psump = ctx.enter_context(tc.tile_pool(name="psum", bufs=1, space="PSUM"))

        ones = constp.tile([P, 1], fp16, name="ones")
        nc.vector.memset(ones, 1.0)

        # one 7-bank PSUM tile: class c's [128,128] tp accumulator lives at
        # the start of bank c-1 (512-f32 bank stride), so start=True bank
        # resets stay per-class, and the final drain is one strided copy.
        BANK = 512
        tpbank = psump.tile([P, 7, BANK], f32, name="tpbank", tag="tpbank")
        psums = [tpbank[:, c - 1, 0:128] for c in range(1, C)]
        # all 7 pred-count accumulators share the 8th PSUM bank
        cntbank = psump.tile([P, 8], f32, name="cntbank", tag="cntbank")
        cnts = [cntbank[:, c - 1 : c] for c in range(1, C)]

        chunk_idx = 0
        for n in range(NB):
            for j in range(NCHUNK):
                ch = []
                for c in range(C):
                    tl = chpool.tile([P, F], f32, name=f"ch{c}", tag=f"ch{c}")
                    nc.sync.dma_start(out=tl, in_=yp[n, c, j])
                    ch.append(tl)
                tt = tpool.tile([P, F], i32, name="t", tag="t")
                nc.sync.dma_start(out=tt, in_=yt[n, j])
                tf = tpool.tile([P, F], fp16, name="tf", tag="tf")
                nc.scalar.copy(out=tf, in_=tt)

                # gt masks first: they only need the labels, so they fill DVE
                # idle time while the channel converts stream on ScalarE
                gts = {}
                for c in range(1, C):
                    gt = gtpool.tile([P, F], fp16, name=f"gt{c}", tag="gt")
                    nc.vector.tensor_scalar(gt, tf, float(c), None, A.is_equal)
                    gts[c] = gt

                # ScalarE: fp32 -> fp16 channel converts
                chf = []
                for c in range(C):
                    tl = hpool.tile([P, F], fp16, name=f"hf{c}", tag=f"hf{c}")
                    nc.scalar.copy(out=tl, in_=ch[c])
                    chf.append(tl)

                # ---- max tree (DVE tensor_tensor fp16: 2x perf mode) ----
                m01 = mtmp.tile([P, F], fp16, name="m01", tag="mt")
                nc.vector.tensor_max(m01, chf[0], chf[1])
                m23 = mtmp.tile([P, F], fp16, name="m23", tag="mt")
                nc.vector.tensor_max(m23, chf[2], chf[3])
                m45 = mtmp.tile([P, F], fp16, name="m45", tag="mt")
                nc.vector.tensor_max(m45, chf[4], chf[5])
                m67 = mtmp.tile([P, F], fp16, name="m67", tag="mt")
                nc.vector.tensor_max(m67, chf[6], chf[7])
                m0123 = mtmp.tile([P, F], fp16, name="m0123", tag="mt")
                nc.vector.tensor_max(m0123, m01, m23)
                m4567 = mtmp.tile([P, F], fp16, name="m4567", tag="mt")
                nc.vector.tensor_max(m4567, m45, m67)
                m = mpool.tile([P, F], fp16, name="m", tag="m")
                nc.vector.tensor_max(m, m0123, m4567)

                # ---- per-class masks + PE tp/count matmuls ----
                for c in range(1, C):
                    pred = maskp.tile([P, F], fp16, name=f"pred{c}", tag="pred")
                    nc.vector.tensor_tensor(pred, chf[c], m, A.is_equal)
                    gt = gts[c]
                    for s in range(NSUB):
                        first = chunk_idx == 0 and s == 0
                        last = chunk_idx == CHUNKS - 1 and s == NSUB - 1
                        nc.tensor.matmul(
                            psums[c - 1],
                            lhsT=pred[:, s * 128 : (s + 1) * 128],
                            rhs=gt[:, s * 128 : (s + 1) * 128],
                            start=first,
                            stop=last,
                            skip_group_check=True,
                        )
                        # cnts share one PSUM bank and start=True resets the
                        # WHOLE bank (measured): only the very first count
                        # matmul may use it; all later chains accumulate.
                        nc.tensor.matmul(
                            cnts[c - 1],
                            lhsT=pred[:, s * 128 : (s + 1) * 128],
                            rhs=ones[:, :],
                            start=first and c == 1,
                            stop=last and c == C - 1,
                            skip_group_check=True,
                        )
                chunk_idx += 1

        # drain: one strided copy picks the [128,128] block out of each bank
        tps = accp.tile([P, 7, 128], f32, name="tps")
        nc.scalar.copy(out=tps, in_=tpbank[:, :, 0:128])
        nc.sync.dma_start(out=tp_out[:, :, :], in_=tps)
        pcs = accp.tile([P, 7], f32, name="pcs")
        nc.scalar.copy(out=pcs, in_=cntbank[:, 0:7])
        nc.sync.dma_start(out=pc_out[:, :], in_=pcs)

    nc.finalize()
    return nc


def _get_bass():
    global _CACHED_NC
    if _CACHED_NC is None:
        _CACHED_NC = build_bass()
    return _CACHED_NC


def make_in_maps(y_true, y_pred):
    yp = np.ascontiguousarray(np.asarray(y_pred, dtype=np.float32))
    yt = np.ascontiguousarray(np.asarray(y_true, dtype=np.int32))
    in_maps = []
    for i in range(N_CORES):
        yps = np.ascontiguousarray(yp[NB * i : NB * (i + 1)]).reshape(NB, C, NCHUNK, P, F)
        yts = np.ascontiguousarray(yt[NB * i : NB * (i + 1)]).reshape(NB, NCHUNK, P, F)
        in_maps.append({"yp": yps, "yt": yts})
    return in_maps


def epilogue(results, y_true):
    """Combine the 8 cores' partial sums into the final dice mean (float32,
    mirroring the reference arithmetic). gt counts come from the labels
    directly (exact)."""
    tp = np.zeros(7, dtype=np.float64)
    pred_cnt = np.zeros(7, dtype=np.float64)
    for r in results:
        # tp_out[m, c, n] = psum_c[m, n]; tp_c = sum_m psum_c[m, m]
        tp += np.einsum("mcm->c", np.asarray(r["tp_out"], dtype=np.float64))
        pred_cnt += np.asarray(r["pc_out"], dtype=np.float64).sum(axis=0)
    gt_cnt = np.bincount(
        np.asarray(y_true, dtype=np.int64).ravel(), minlength=8
    )[1:].astype(np.float64)

    # dice = (2tp + eps) / (2tp + fp + fn + eps), and
    # 2tp + fp + fn = pred_cnt + gt_cnt
    tp32 = tp.astype(np.float32)
    denom = (pred_cnt + gt_cnt).astype(np.float32)
    eps = np.float32(EPS)
    two = np.float32(2.0)
    dice = (two * tp32 + eps) / (denom + eps)
    return np.asarray(np.mean(dice, dtype=np.float32), dtype=np.float32)


def kernel(**inputs):
    from concourse.bass_utils import run_bass_kernel_spmd

    nc = _get_bass()
    in_maps = make_in_maps(inputs["y_true"], inputs["y_pred"])
    res = run_bass_kernel_spmd(nc, in_maps, core_ids=list(range(N_CORES)))
    return epilogue(res.results, inputs["y_true"])


if __name__ == "__main__":
    # smoke test with random data
    rng = np.random.default_rng(0)
    y_true = rng.integers(0, C, size=(16, 512, 512)).astype(np.int32)
    y_pred = rng.standard_normal((16, C, 512, 512)).astype(np.float32)
    out = kernel(y_true=y_true, y_pred=y_pred)
    print("kernel output:", out)
